# revision 1
# baseline (speedup 1.0000x reference)
"""GRU sequence encoder (DiscSeqRNNEncoder) for 8x TRN2 NeuronCores.

Strategy: pure data-parallel over the batch (1024 rows/core).  On-device
everything lives in "transposed" layout [hidden/gate on partitions, batch on
free] so the recurrent state never needs a transpose.  Host-side prep does
the embedding gather into a transposed fp16 stream with an appended ones-row
(so the PE matmuls fold all biases in), plus the n-gate input projection
table gather.  Per time step the PE accumulates r/z pre-activations
(input + recurrent halves) directly in PSUM, ScalarE applies one sigmoid
over r||z and a tanh, and DVE/GPSIMD do the five remaining elementwise ops
(fp16, with a fused scalar_tensor_tensor for r*(hn+b_hh_n)).  Two
half-batch chains (512 each) pipeline through the engines to hide the
serial dependency of the recurrence.

All constants arrive in ONE packed DMA and each step's inputs (embeddings
stream + n-gate input projections) in ONE DMA, prefetched two steps ahead.
Multi-wait legalization (one sync wait per hardware instruction) is handled
by Bacc.compile()'s generate_event_semaphores pass.
"""

import numpy as np

import concourse.bass as bass
import concourse.tile as tile
from concourse import bacc
from concourse import mybir
from concourse.bass_utils import run_bass_kernel_spmd

F16 = mybir.dt.float16
F32 = mybir.dt.float32

B, L = 8192, 64
NV, E, H, OUT = 1000, 64, 128, 128
N_CORES = 8
BL = B // N_CORES          # batch rows per core
EA = 80                    # embed (64) + ones row (1) padded to 80 partitions
NCONST = 1025              # packed const block free size

_BUILD_CACHE = {}


def build_nc(n_steps=L, bl=BL, n_chains=2, prefetch=4, repeats=1,
             psrz_bufs=3, pshn_bufs=2, gates_bufs=6, h_bufs=2,
             merged_sigma=False, zh_on_dve="alt", zc_alt=False, nzc_alt=False,
             path_prio=None, stt_split=False, split_rz=False, u_alt=False):
    """Build the single-core Bass/Tile program (SPMD across 8 cores).

    repeats > 1 re-runs the whole recurrence (for differential wall-clock
    timing); numerics then chain h across repeats, which is fine for timing.
    """
    bc = bl // n_chains  # batch per chain
    nc = bacc.Bacc("TRN2", target_bir_lowering=False, debug=False)

    st_d = nc.dram_tensor("stream", [n_steps, H, 2 * bl], F16,
                          kind="ExternalInput").ap()
    cb_d = nc.dram_tensor("consts", [H, NCONST], F16, kind="ExternalInput").ap()
    out_d = nc.dram_tensor("out", [bl, OUT], F32, kind="ExternalOutput").ap()

    AF = mybir.ActivationFunctionType
    OP = mybir.AluOpType
    import contextlib


    with tile.TileContext(nc) as tc:
        cpool = tc.alloc_tile_pool(name="consts", bufs=1)
        stpool = tc.alloc_tile_pool(name="stream", bufs=prefetch)
        hpool = tc.alloc_tile_pool(name="hstate", bufs=h_bufs)
        gpool = tc.alloc_tile_pool(name="gates", bufs=gates_bufs)
        psrz = tc.alloc_tile_pool(name="psrz", bufs=psrz_bufs, space="PSUM")
        psz2 = (tc.alloc_tile_pool(name="psz2", bufs=psrz_bufs, space="PSUM")
                if split_rz else None)
        pshn = tc.alloc_tile_pool(name="pshn", bufs=pshn_bufs, space="PSUM")

        cb = cpool.tile([H, NCONST], F16, name="cb_sb")
        nc.sync.dma_start(cb[:], cb_d[:])
        w_r = cb[0:EA, 0:128]
        w_z = cb[0:EA, 128:256]
        whh_r = cb[:, 256:384]
        whh_z = cb[:, 384:512]
        whh_n = cb[:, 512:640]
        woutT = cb[:, 640:768]
        bhn = cb[:, 768:769]
        ones1 = cb[0:1, 769:897]
        bout1 = cb[0:1, 897:1025]

        # initial hidden state = 0
        h = []
        for c in range(n_chains):
            h0 = hpool.tile([H, bc], F16, name=f"h0_{c}", tag=f"h{c}")
            nc.gpsimd.memset(h0[:], 0.0)
            h.append(h0)

        # dummy sigmoid on the zeroed h0 pulls the ACT table load
        # (~2.7us) into the prologue, hidden behind the input DMAs
        warm = gpool.tile([H, 8], F16, name="warm_sb", tag="warm")
        nc.scalar.activation(warm[:], h[0][:, 0:8], AF.Sigmoid)

        # stream prefetch, 2 steps ahead
        streams = {}

        def issue_stream(t):
            st = stpool.tile([H, 2 * bl], F16, name="st_t", tag="st")
            dma = nc.sync.dma_start(st[:], st_d[t % n_steps])
            streams[t] = (st, dma)

        issue_stream(0)
        total_steps = n_steps * repeats
        if total_steps > 1:
            issue_stream(1)

        for t in range(total_steps):
            if t + 2 < total_steps:
                issue_stream(t + 2)
            st, st_dma = streams.pop(t)
            et = st[0:EA, 0:bl]
            xn = st[:, bl:2 * bl]

            for c in range(n_chains):
                ecol = et[:, c * bc:(c + 1) * bc]
                xcol = xn[:, c * bc:(c + 1) * bc]

                if split_rz:
                    r_ps = psrz.tile([H, bc], F32, name="r_ps", tag="rz")
                    z_ps = psz2.tile([H, bc], F32, name="z_ps", tag="z2")
                else:
                    rz_ps = psrz.tile([H, 2 * bc], F32, name="rz_ps", tag="rz")
                    r_ps = rz_ps[:, 0:bc]
                    z_ps = rz_ps[:, bc:2 * bc]
                hn_ps = pshn.tile([H, bc], F32, name="hn_ps", tag="hn")

                # pre-activations: input half first (no dependence on h),
                # then recurrent half accumulates on top
                nc.tensor.matmul(r_ps[:], w_r, ecol,
                                 start=True, stop=False)
                nc.tensor.matmul(z_ps[:], w_z, ecol,
                                 start=True, stop=False)
                nc.tensor.matmul(r_ps[:], whh_r, h[c][:],
                                 start=False, stop=True)
                nc.tensor.matmul(z_ps[:], whh_z, h[c][:],
                                 start=False, stop=True)
                nc.tensor.matmul(hn_ps[:], whh_n, h[c][:],
                                 start=True, stop=True)

                # Critical path per step is h -> hg_r MM -> sigmoid(r) ->
                # tt -> u -> tanh -> nzc -> h_new.  Everything z-related is
                # off-path: sigmoid(z) feeds zc = 1-z and zh = z*h, both on
                # GPSIMD, so only two DVE ops follow the tanh.
                prio = (lambda: tc.high_priority(offset=path_prio)) \
                    if path_prio else contextlib.nullcontext
                rz = gpool.tile([H, 2 * bc], F16, name="rz_sb", tag="rz_sb")
                tt = gpool.tile([H, bc], F16, name="tt_sb", tag="tt")
                u = gpool.tile([H, bc], F16, name="u_sb", tag="u")
                n_sb = gpool.tile([H, bc], F16, name="n_sb", tag="n")
                with prio():
                    if merged_sigma:
                        nc.scalar.activation(rz[:], r_ps[:], AF.Sigmoid)  # merged needs contiguity; only valid when not split_rz
                    else:
                        nc.scalar.activation(rz[:, 0:bc], r_ps[:],
                                             AF.Sigmoid)
                    # tt = (hn + b_hh_n) * r
                    if stt_split:
                        hb = bc // 2
                        nc.vector.scalar_tensor_tensor(
                            tt[:, 0:hb], hn_ps[:, 0:hb], bhn, rz[:, 0:hb],
                            op0=OP.add, op1=OP.mult)
                        nc.vector.scalar_tensor_tensor(
                            tt[:, hb:bc], hn_ps[:, hb:bc], bhn,
                            rz[:, hb:bc], op0=OP.add, op1=OP.mult)
                    else:
                        nc.vector.scalar_tensor_tensor(
                            tt[:], hn_ps[:], bhn, rz[:, 0:bc],
                            op0=OP.add, op1=OP.mult)
                    if u_alt and c == 1:
                        nc.gpsimd.tensor_add(u[:], tt[:], xcol)
                    else:
                        nc.vector.tensor_add(u[:], tt[:], xcol)
                    nc.scalar.activation(n_sb[:], u[:], AF.Tanh)

                if not merged_sigma:
                    nc.scalar.activation(rz[:, bc:2 * bc], z_ps[:],
                                         AF.Sigmoid)
                zc = gpool.tile([H, bc], F16, name="zc_sb", tag="zc")
                if zc_alt and c == 0:
                    nc.vector.tensor_scalar(zc[:], rz[:, bc:2 * bc], -1.0, 1.0,
                                            OP.mult, OP.add)
                else:
                    nc.gpsimd.tensor_scalar(zc[:], rz[:, bc:2 * bc], -1.0, 1.0,
                                            OP.mult, OP.add)
                zh = gpool.tile([H, bc], F16, name="zh_sb", tag="zh")
                if zh_on_dve is True or (zh_on_dve == "alt" and c == 1) \
                        or (zh_on_dve == "alt0" and c == 0):
                    nc.vector.tensor_mul(zh[:], rz[:, bc:2 * bc], h[c][:])
                else:
                    nc.gpsimd.tensor_mul(zh[:], rz[:, bc:2 * bc], h[c][:])

                # h_new = n*(1-z) + z*h
                nzc = gpool.tile([H, bc], F16, name="nzc_sb", tag="nzc")
                h_new = hpool.tile([H, bc], F16, name=f"hn_{c}", tag=f"h{c}")
                with prio():
                    if nzc_alt and c == 0:
                        nc.gpsimd.tensor_mul(nzc[:], n_sb[:], zc[:])
                    else:
                        nc.vector.tensor_mul(nzc[:], n_sb[:], zc[:])
                    nc.vector.tensor_add(h_new[:], nzc[:], zh[:])
                h[c] = h_new

        # output head: out[b, :] = h_last[:, b] . W_outT + b_out
        for c in range(n_chains):
            for bt in range(bc // H):
                o_ps = pshn.tile([H, OUT], F32, name="o_ps", tag="hn")
                lhs = h[c][:, bt * H:(bt + 1) * H]
                nc.tensor.matmul(o_ps[:], lhs, woutT,
                                 start=True, stop=False)
                nc.tensor.matmul(o_ps[:], ones1, bout1,
                                 start=False, stop=True)
                o_sb = gpool.tile([H, OUT], F32, name="o_sb", tag="osb")
                nc.scalar.activation(o_sb[:], o_ps[:], AF.Copy)
                r0 = c * bc + bt * H
                nc.sync.dma_start(out_d[r0:r0 + H], o_sb[:])

        pools = [pshn] + ([psz2] if split_rz else []) + [psrz, gpool, hpool, stpool, cpool]
        for p in pools:
            p.release()

    nc.compile()
    return nc


def _host_prep(inputs, n_steps=L, bl=BL):
    """Shared (weights) + per-core (streams) host-side layout prep."""
    x = np.asarray(inputs["x"]).astype(np.int64)
    embed = np.asarray(inputs["embed"], dtype=np.float32)
    W_ih = np.asarray(inputs["W_ih"], dtype=np.float32)
    W_hh = np.asarray(inputs["W_hh"], dtype=np.float32)
    b_ih = np.asarray(inputs["b_ih"], dtype=np.float32)
    b_hh = np.asarray(inputs["b_hh"], dtype=np.float32)
    W_out = np.asarray(inputs["W_out"], dtype=np.float32)
    b_out = np.asarray(inputs["b_out"], dtype=np.float32)

    def aug_w(g):
        # lhsT [EA, H]: rows 0:64 = W_ih[g].T, row 64 = combined bias, rest 0
        w = np.zeros((EA, H), np.float16)
        w[:E] = W_ih[g * H:(g + 1) * H].T.astype(np.float16)
        w[E] = (b_ih[g * H:(g + 1) * H] + b_hh[g * H:(g + 1) * H]).astype(np.float16)
        return w

    cb = np.zeros((H, NCONST), np.float16)
    cb[0:EA, 0:128] = aug_w(0)
    cb[0:EA, 128:256] = aug_w(1)
    cb[:, 256:384] = W_hh[0:H].T.astype(np.float16)
    cb[:, 384:512] = W_hh[H:2 * H].T.astype(np.float16)
    cb[:, 512:640] = W_hh[2 * H:3 * H].T.astype(np.float16)
    cb[:, 640:768] = W_out.T.astype(np.float16)
    cb[:, 768] = b_hh[2 * H:3 * H].astype(np.float16)
    cb[0, 769:897] = 1.0
    cb[0, 897:1025] = b_out.astype(np.float16)

    # embed table with ones column for the bias rows of the aug weights
    T_aug = np.zeros((NV, EA), np.float16)
    T_aug[:, :E] = embed.astype(np.float16)
    T_aug[:, E] = 1.0
    # n-gate input projection table (bias folded in)
    G_n = (embed @ W_ih[2 * H:3 * H].T + b_ih[2 * H:3 * H]).astype(np.float16)

    per_core = []
    n_cores = x.shape[0] // bl
    for i in range(n_cores):
        xc = x[i * bl:(i + 1) * bl, :n_steps]              # [bl, n_steps]
        stream = np.zeros((n_steps, H, 2 * bl), np.float16)
        stream[:, 0:EA, 0:bl] = T_aug[xc].transpose(1, 2, 0)
        stream[:, :, bl:2 * bl] = G_n[xc].transpose(1, 2, 0)
        per_core.append({"stream": stream, "consts": cb})
    return per_core


def _run(inputs, trace=False, **kw):
    if "full" not in _BUILD_CACHE:
        _BUILD_CACHE["full"] = build_nc()
    nc = _BUILD_CACHE["full"]
    in_maps = _host_prep(inputs)
    res = run_bass_kernel_spmd(nc, in_maps, list(range(N_CORES)), trace=trace, **kw)
    out = np.concatenate([res.results[i]["out"] for i in range(N_CORES)], axis=0)
    return out.astype(np.float32), res


def kernel(**inputs) -> np.ndarray:
    out, _ = _run(inputs)
    return out

